# revision 2
# baseline (speedup 1.0000x reference)
"""Trainium2 Bass kernel for nn_AttnResLayer (sparse_attention).

Computes, for V [N=12, B=4, T=2048, D=1024] fp32:
  K = rmsnorm(V) * norm_weight
  logits[n,b,t] = dot(w_l, K[n,b,t,:])
  alpha = softmax(logits, axis=n)
  out[b,t,d] = sum_n alpha[n,b,t] * V[n,b,t,d]

Sharding: T split across 8 cores (256 tokens/core per b); w_l/norm_weight
replicated (folded into one weight row host-side). No collectives.

fp16 transport: V is cast to fp16 on the host (and the output is returned
as fp16, upcast on the host), halving HBM traffic vs fp32 — the op only
needs rel_err < 2e-2 and the fp16 pipeline measures ~1.1e-2 end to end.
All reductions accumulate in fp32 on-chip.

Engine balance per 128-token chunk (12 slices of [128, 1024] fp16):
  - ss  (sum_d V^2): 9 slices on ACT (Square + fused fp32 accum, 1225ns),
    3 slices on DVE as TT(v,v)@2x + tensor_scalar-sum@4x (921ns)
  - dot (sum_d w*V): TT(v, wb)@2x + tensor_scalar-sum@4x on DVE for 9
    slices; the TT multiply for 3 slices runs on the otherwise-idle Pool
    engine (Q7 software TT, 2128ns) with the cheap @4x sum on DVE
  - softmax over n and rms = exp(-0.5*ln(ss/D+eps)) on ACT/DVE smalls;
    normalization folded into the diag build (tensor_scalar, two scalar
    ptrs, @4x on fp16)
  - out = sum_n diag(alpha_n) @ V_n on PE (fp16 matmul, 1 cycle/row),
    PSUM fp32; drain PSUM->SBUF fp16 split ACT(Copy)/DVE(tensor_copy)
  - diags split Pool (TT broadcast vs id) / DVE as a balance knob
All activations pinned to the one table set holding ln+exp+square+copy.
DMA: per-slice loads on SP in slice order (Pool's slices first); ALL
output stores issue after the last load so the store traffic (fp16,
~5.8us) covers the final chunk's compute tail. PE p-state kept warm with
pacing matmuls between MAC bursts.
"""

import numpy as np
from contextlib import ExitStack

import concourse.bass as bass
import concourse.bacc as bacc
import concourse.tile as tile
from concourse import mybir
from concourse.bass_utils import run_bass_kernel_spmd


def _pinned_tables(arch, _orig=bacc.get_activation_tables):
    tables = _orig(arch)
    keep = "natural_log_exp_and_others"
    return {k: (v if k == keep else set()) for k, v in tables.items()}


N, B, T, D = 12, 4, 2048, 1024
NCORES = 8
TSH = T // NCORES
P = 128
NCHUNK = TSH // P
NCK = B * NCHUNK
EPS = 1e-6
FP32 = mybir.dt.float32
FP16 = mybir.dt.float16
AF = mybir.ActivationFunctionType
ALU = mybir.AluOpType

# engine assignment knobs (per 12-slice chunk)
POOL_DOT = (0, 1, 2)          # dot-product TT multiplies on Pool
DVE_SQ = (9, 10, 11)          # squares on DVE (TT+sum) instead of ACT
POOL_DIAG = tuple(range(12))  # diag builds on Pool (rest on DVE)
ACT_DRAIN_COLS = 640          # PSUM drain split: [0,that) ACT, rest DVE


def _build_nc() -> bacc.Bacc:
    nc = bacc.Bacc("TRN2", target_bir_lowering=False, debug=False,
                   num_devices=NCORES)
    v_in = nc.dram_tensor("v", [N, B, TSH, D], FP16, kind="ExternalInput").ap()
    wb_in = nc.dram_tensor("wb", [2, D // 2], FP16, kind="ExternalInput").ap()
    ones_in = nc.dram_tensor("ones", [2, P], FP16, kind="ExternalInput").ap()
    out_d = nc.dram_tensor("out", [B, TSH, D], FP16, kind="ExternalOutput").ap()

    orig_tables = bacc.get_activation_tables
    bacc.get_activation_tables = _pinned_tables
    try:
        _build_body(nc, v_in, wb_in, ones_in, out_d)
    finally:
        bacc.get_activation_tables = orig_tables
    return nc


def _build_body(nc, v_in, wb_in, ones_in, out_d):
    with tile.TileContext(nc) as tc, ExitStack() as ctx:
        const_pool = ctx.enter_context(tc.tile_pool(name="const", bufs=1))
        v_pool = ctx.enter_context(tc.tile_pool(name="vp", bufs=2))
        scr_pool = ctx.enter_context(tc.tile_pool(name="scr", bufs=1))
        scrp_pool = ctx.enter_context(tc.tile_pool(name="scrp", bufs=2))
        small_pool = ctx.enter_context(tc.tile_pool(name="small", bufs=4))
        diag_pool = ctx.enter_context(tc.tile_pool(name="diag", bufs=16))
        psum_pool = ctx.enter_context(
            tc.tile_pool(name="accp", bufs=2, space="PSUM"))
        warm_pool = ctx.enter_context(
            tc.tile_pool(name="warmp", bufs=1, space="PSUM"))
        out_pool = ctx.enter_context(tc.tile_pool(name="outp", bufs=8))

        eps_t = const_pool.tile([P, 1], FP32, name="eps_t")
        nc.vector.memset(eps_t[:], EPS)
        # broadcast the folded weight row [1, D] to [128, D] fp16 on-chip
        # via a ones-column matmul (PSUM fp32 -> copy to fp16)
        wbsm = const_pool.tile([33, D // 2], FP16, name="wbsm")
        nc.scalar.dma_start(wbsm[0:33:32, :], wb_in[:])
        ones_t = const_pool.tile([33, P], FP16, name="ones_t")
        nc.scalar.dma_start(ones_t[0:33:32, :], ones_in[:])
        id16 = const_pool.tile([P, P], FP16, name="id16")
        wb_t = const_pool.tile([P, D], FP16, name="wb_t")
        for h in range(2):
            wbp = psum_pool.tile([P, 512], FP32, name="wbp", tag="accl")
            nc.tensor.matmul(wbp[:], ones_t[h * 32:h * 32 + 1, :],
                             wbsm[h * 32:h * 32 + 1, :],
                             start=True, stop=True)
            nc.vector.tensor_copy(wb_t[:, h * 512:(h + 1) * 512], wbp[:])
        scr_act = scr_pool.tile([P, D], FP16, name="scr_act")
        scr_dve = scr_pool.tile([P, D], FP16, name="scr_dve")

        stores = []
        for ci in range(NCK):
            b, c = divmod(ci, NCHUNK)
            t0 = c * P
            last = ci == NCK - 1
            vslices = []
            for q in range(N):
                vt = v_pool.tile([P, D], FP16, name=f"vs{q}", tag=f"vs{q}")
                nc.sync.dma_start(vt[:], v_in[q, b, t0:t0 + P, :])
                vslices.append(vt)
            if ci == 0:
                # fp16 identity generated by Pool (memset + keep diagonal)
                nc.gpsimd.memset(id16[:], 1.0)
                nc.gpsimd.affine_select(out=id16[:], in_=id16[:],
                                        pattern=[[1, P]],
                                        compare_op=ALU.is_equal, fill=0.0,
                                        base=0, channel_multiplier=-1)
            vts = [vslices[q][:] for q in range(N)]

            # PE p-state pacing: one throwaway matmul per arriving slice
            warm_ps = warm_pool.tile([P, 512], FP32, name="warm_ps", tag="wp")
            if ci > 0:
                for q in range(N):
                    nc.tensor.matmul(warm_ps[:], id16[:],
                                     vts[q][:, 0:512],
                                     start=True, stop=True)

            ss = small_pool.tile([P, N], FP32, name="ss", tag="ss")
            dot = small_pool.tile([P, N], FP32, name="dot", tag="dot")

            # Pool dot-product multiplies for its slices (issued first,
            # on the earliest-arriving slices)
            pool_prods = {}
            for q in POOL_DOT:
                sp = scrp_pool.tile([P, D], FP16, name="scrp", tag="scrp")
                nc.gpsimd.tensor_tensor(out=sp[:], in0=vts[q], in1=wb_t[:],
                                        op=ALU.mult)
                pool_prods[q] = sp

            # ACT squares
            for q in range(N):
                if q in DVE_SQ:
                    continue
                nc.scalar.activation(scr_act[:], vts[q], AF.Square,
                                     accum_out=ss[:, q:q + 1])
            # DVE work, in slice order: dots (TT+sum) + DVE squares
            for q in range(N):
                if q in POOL_DOT:
                    nc.vector.tensor_scalar(
                        out=pool_prods[q][:], in0=pool_prods[q][:],
                        scalar1=1.0, scalar2=0.0, op0=ALU.mult, op1=ALU.add,
                        accum_out=dot[:, q:q + 1])
                else:
                    nc.vector.tensor_tensor(out=scr_dve[:], in0=vts[q],
                                            in1=wb_t[:], op=ALU.mult)
                    nc.vector.tensor_scalar(
                        out=scr_dve[:], in0=scr_dve[:],
                        scalar1=1.0, scalar2=0.0, op0=ALU.mult, op1=ALU.add,
                        accum_out=dot[:, q:q + 1])
                if q in DVE_SQ:
                    nc.vector.tensor_tensor(out=scr_dve[:], in0=vts[q],
                                            in1=vts[q], op=ALU.mult)
                    nc.vector.tensor_scalar(
                        out=scr_dve[:], in0=scr_dve[:],
                        scalar1=1.0, scalar2=0.0, op0=ALU.mult, op1=ALU.add,
                        accum_out=ss[:, q:q + 1])

            if last:
                # keep the PE p-state hot across the reduction->softmax gap
                for _ in range(13):
                    nc.tensor.matmul(warm_ps[:], id16[:],
                                     vts[N - 1][:, 0:512],
                                     start=True, stop=True)

            # rms = (mean(V^2) + eps)^-0.5 = exp(-0.5*ln(ss/D + eps))
            u = small_pool.tile([P, N], FP32, name="u", tag="u")
            nc.scalar.activation(u[:], ss[:], AF.Ln, bias=eps_t[:, 0:1],
                                 scale=1.0 / D)
            rms = small_pool.tile([P, N], FP32, name="rms", tag="rms")
            nc.scalar.activation(rms[:], u[:], AF.Exp, scale=-0.5)
            logits = small_pool.tile([P, N], FP32, name="logits", tag="lg")
            nc.vector.tensor_mul(logits[:], dot[:], rms[:])

            negmax = small_pool.tile([P, 1], FP32, name="negmax", tag="nm")
            nc.vector.tensor_reduce(negmax[:], logits[:],
                                    axis=mybir.AxisListType.X,
                                    op=ALU.max, negate=True)
            aexp = small_pool.tile([P, N], FP32, name="aexp", tag="ax")
            sumexp = small_pool.tile([P, 1], FP32, name="sumexp", tag="se")
            nc.scalar.activation(aexp[:], logits[:], AF.Exp,
                                 bias=negmax[:, 0:1], accum_out=sumexp[:])
            recip = small_pool.tile([P, 1], FP32, name="recip", tag="rc")
            nc.vector.reciprocal(recip[:], sumexp[:])
            # normalized alpha for the Pool diag path
            anorm = small_pool.tile([P, N], FP32, name="anorm", tag="an")
            nc.vector.tensor_scalar(out=anorm[:], in0=aexp[:],
                                    scalar1=recip[:, 0:1], scalar2=1.0,
                                    op0=ALU.mult, op1=ALU.mult)

            dgs = []
            for n in range(N):
                dg = diag_pool.tile([P, P], FP16, name="dg", tag="dg")
                if n in POOL_DIAG:
                    nc.gpsimd.tensor_tensor(
                        out=dg[:], in0=id16[:],
                        in1=anorm[:, n:n + 1].broadcast_to([P, P]),
                        op=ALU.mult)
                else:
                    nc.vector.tensor_scalar(out=dg[:], in0=id16[:],
                                            scalar1=aexp[:, n:n + 1],
                                            scalar2=recip[:, 0:1],
                                            op0=ALU.mult, op1=ALU.mult)
                dgs.append(dg)

            # out[t, d] = sum_n alpha[n, t] * V_n[t, d] on TensorE
            out_sb = out_pool.tile([P, D], FP16, name="out_sb", tag="ot")
            if not last:
                acc = psum_pool.tile([P, D], FP32, name="acc", tag="acc")
                for h in range(2):
                    for n in range(N):
                        nc.tensor.matmul(acc[:, h * 512:(h + 1) * 512],
                                         dgs[n][:],
                                         vts[n][:, h * 512:(h + 1) * 512],
                                         start=(n == 0), stop=(n == N - 1))
                ac = ACT_DRAIN_COLS
                with tc.high_priority(offset=-100):
                    if ac > 0:
                        nc.scalar.activation(out_sb[:, 0:ac], acc[:, 0:ac],
                                             AF.Copy)
                    if ac < D:
                        nc.vector.tensor_copy(out_sb[:, ac:D], acc[:, ac:D])
                stores.append((out_d[b, t0:t0 + P, :], out_sb[:]))
            else:
                # final chunk: drain in pieces on separate PSUM tiles so the
                # kernel tail past the last MAC is short
                for a0, a1 in ((0, 512), (512, 768), (768, 1024)):
                    w = a1 - a0
                    accl = psum_pool.tile([P, 512], FP32, name="accl",
                                          tag="accl")
                    for n in range(N):
                        nc.tensor.matmul(accl[:, 0:w], dgs[n][:],
                                         vts[n][:, a0:a1],
                                         start=(n == 0), stop=(n == N - 1))
                    if a0 == 0:
                        nc.scalar.activation(out_sb[:, a0:a1], accl[:, 0:w],
                                             AF.Copy)
                    else:
                        nc.vector.tensor_copy(out_sb[:, a0:a1], accl[:, 0:w])
                    stores.append((out_d[b, t0:t0 + P, a0:a1],
                                   out_sb[:, a0:a1]))

        # all stores issue after the last load in SP program order
        for dst, src in stores:
            nc.sync.dma_start(dst, src)
    nc.compile()
    return nc


_NC = None


def _get_nc() -> bacc.Bacc:
    global _NC
    if _NC is None:
        _NC = _build_nc()
    return _NC


def _make_in_maps(V, w_l, norm_weight):
    V16 = np.asarray(V).astype(np.float16)
    w = np.asarray(w_l, np.float32) * np.asarray(norm_weight, np.float32)
    wb = np.ascontiguousarray(w.astype(np.float16).reshape(2, D // 2))
    ones = np.ones((2, P), dtype=np.float16)
    in_maps = []
    for c in range(NCORES):
        vs = np.ascontiguousarray(V16[:, :, c * TSH:(c + 1) * TSH, :])
        in_maps.append({"v": vs, "wb": wb, "ones": ones})
    return in_maps


def _run(in_maps, trace=False, **kwargs):
    return run_bass_kernel_spmd(_get_nc(), in_maps, list(range(NCORES)),
                                trace=trace, **kwargs)


def kernel(V, w_l, norm_weight):
    res = _run(_make_in_maps(V, w_l, norm_weight))
    outs = [res.results[i]["out"] for i in range(NCORES)]
    return np.concatenate(outs, axis=1).astype(np.float32)


# revision 3
# speedup vs baseline: 1.0251x; 1.0251x over previous
"""Trainium2 Bass kernel for nn_AttnResLayer (sparse_attention).

Computes, for V [N=12, B=4, T=2048, D=1024] fp32:
  K = rmsnorm(V) * norm_weight
  logits[n,b,t] = dot(w_l, K[n,b,t,:])
  alpha = softmax(logits, axis=n)
  out[b,t,d] = sum_n alpha[n,b,t] * V[n,b,t,d]

Sharding: T split across 8 cores (256 tokens/core per b); w_l/norm_weight
replicated (folded into one weight row host-side). No collectives.

fp16 transport: V is cast to fp16 on the host (and the output returned as
fp16, upcast on the host), halving HBM traffic vs fp32 — the op only needs
rel_err < 2e-2 and the fp16 pipeline measures ~1.1e-2 end to end. All
reductions accumulate in fp32 on-chip.

Two-stage software pipeline over 128-token chunks: during chunk i's load +
reduction phase (R), chunk i-1's softmax smalls, diag builds, MAC and PSUM
drain (S) run interleaved on each engine's in-order queue, with the smalls
FIRST in every queue so the diags/MAC start ~1us into the period instead
of after the period's heavy work.

Per-chunk engine balance (12 slices of [128, 1024] fp16):
  ACT : 9 squares (Square + fused fp32 accum, 1225ns) + ln/exp smalls +
        640 cols of the PSUM drain (Copy)
  DVE : 9 dot TT(v,wb)@2x + 12 tensor_scalar-sums@4x + 3 square TT+sum
        pairs + softmax smalls + 4 diags@4x + 384 cols of drain
  Pool: 3 dot TT multiplies (Q7 software) + 8 diag TTs (id x alpha bcast)
  PE  : 24 MAC matmuls (fp16, 1 cycle/row) + p-state pacing warms
All activations pinned to the table set holding ln+exp+square+copy.
All output stores issue after the last load so the fp16 store traffic
covers the final chunk's compute tail.
"""

import numpy as np
from contextlib import ExitStack

import concourse.bass as bass
import concourse.bacc as bacc
import concourse.tile as tile
from concourse import mybir
from concourse.bass_utils import run_bass_kernel_spmd


def _pinned_tables(arch, _orig=bacc.get_activation_tables):
    tables = _orig(arch)
    keep = "natural_log_exp_and_others"
    return {k: (v if k == keep else set()) for k, v in tables.items()}


N, B, T, D = 12, 4, 2048, 1024
NCORES = 8
TSH = T // NCORES
P = 128
NCHUNK = TSH // P
NCK = B * NCHUNK
EPS = 1e-6
FP32 = mybir.dt.float32
FP16 = mybir.dt.float16
AF = mybir.ActivationFunctionType
ALU = mybir.AluOpType

# engine assignment knobs (per 12-slice chunk)
POOL_DOT = (0, 1, 2)     # dot-product TT multiplies on Pool
DVE_SQ = (9, 10, 11)     # squares on DVE (TT+sum) instead of ACT
N_POOL_DIAG = 8          # diags 0..7 on Pool, rest on DVE
ACT_DRAIN_COLS = 640     # PSUM drain split: [0,that) ACT, rest DVE


def _build_nc() -> bacc.Bacc:
    nc = bacc.Bacc("TRN2", target_bir_lowering=False, debug=False,
                   num_devices=NCORES)
    v_in = nc.dram_tensor("v", [N, B, TSH, D], FP16, kind="ExternalInput").ap()
    wb_in = nc.dram_tensor("wb", [2, D // 2], FP16, kind="ExternalInput").ap()
    ones_in = nc.dram_tensor("ones", [2, P], FP16, kind="ExternalInput").ap()
    out_d = nc.dram_tensor("out", [B, TSH, D], FP16, kind="ExternalOutput").ap()

    orig_tables = bacc.get_activation_tables
    bacc.get_activation_tables = _pinned_tables
    try:
        _build_body(nc, v_in, wb_in, ones_in, out_d)
    finally:
        bacc.get_activation_tables = orig_tables
    return nc


def _build_body(nc, v_in, wb_in, ones_in, out_d):
    with tile.TileContext(nc) as tc, ExitStack() as ctx:
        const_pool = ctx.enter_context(tc.tile_pool(name="const", bufs=1))
        v_pool = ctx.enter_context(tc.tile_pool(name="vp", bufs=3))
        scr_pool = ctx.enter_context(tc.tile_pool(name="scr", bufs=1))
        scrp_pool = ctx.enter_context(tc.tile_pool(name="scrp", bufs=2))
        small_pool = ctx.enter_context(tc.tile_pool(name="small", bufs=3))
        diag_pool = ctx.enter_context(tc.tile_pool(name="diag", bufs=16))
        psum_pool = ctx.enter_context(
            tc.tile_pool(name="accp", bufs=2, space="PSUM"))
        warm_pool = ctx.enter_context(
            tc.tile_pool(name="warmp", bufs=1, space="PSUM"))
        out_pool = ctx.enter_context(tc.tile_pool(name="outp", bufs=8))

        eps_t = const_pool.tile([P, 1], FP32, name="eps_t")
        nc.vector.memset(eps_t[:], EPS)
        # broadcast the folded weight row [1, D] to [128, D] fp16 on-chip
        # via a ones-column matmul (PSUM fp32 -> copy to fp16)
        wbsm = const_pool.tile([33, D // 2], FP16, name="wbsm")
        nc.scalar.dma_start(wbsm[0:33:32, :], wb_in[:])
        ones_t = const_pool.tile([33, P], FP16, name="ones_t")
        nc.scalar.dma_start(ones_t[0:33:32, :], ones_in[:])
        id16 = const_pool.tile([P, P], FP16, name="id16")
        wb_t = const_pool.tile([P, D], FP16, name="wb_t")
        for h in range(2):
            wbp = psum_pool.tile([P, 512], FP32, name="wbp", tag="accl")
            nc.tensor.matmul(wbp[:], ones_t[h * 32:h * 32 + 1, :],
                             wbsm[h * 32:h * 32 + 1, :],
                             start=True, stop=True)
            nc.vector.tensor_copy(wb_t[:, h * 512:(h + 1) * 512], wbp[:])
        scr_act = scr_pool.tile([P, D], FP16, name="scr_act")
        scr_dve = scr_pool.tile([P, D], FP16, name="scr_dve")

        stores = []
        pend = None  # reduction results of the previous chunk

        def softmax_smalls(st):
            """ACT+DVE small-op chain: ss,dot -> normalized alpha."""
            u = small_pool.tile([P, N], FP32, name="u", tag="u")
            nc.scalar.activation(u[:], st["ss"][:], AF.Ln,
                                 bias=eps_t[:, 0:1], scale=1.0 / D)
            rms = small_pool.tile([P, N], FP32, name="rms", tag="rms")
            nc.scalar.activation(rms[:], u[:], AF.Exp, scale=-0.5)
            logits = small_pool.tile([P, N], FP32, name="lg", tag="lg")
            nc.vector.tensor_mul(logits[:], st["dot"][:], rms[:])
            negmax = small_pool.tile([P, 1], FP32, name="nm", tag="nm")
            nc.vector.tensor_reduce(negmax[:], logits[:],
                                    axis=mybir.AxisListType.X,
                                    op=ALU.max, negate=True)
            aexp = small_pool.tile([P, N], FP32, name="ax", tag="ax")
            sumexp = small_pool.tile([P, 1], FP32, name="se", tag="se")
            nc.scalar.activation(aexp[:], logits[:], AF.Exp,
                                 bias=negmax[:, 0:1], accum_out=sumexp[:])
            recip = small_pool.tile([P, 1], FP32, name="rc", tag="rc")
            nc.vector.reciprocal(recip[:], sumexp[:])
            anorm = small_pool.tile([P, N], FP32, name="an", tag="an")
            nc.vector.tensor_scalar(out=anorm[:], in0=aexp[:],
                                    scalar1=recip[:, 0:1], scalar2=1.0,
                                    op0=ALU.mult, op1=ALU.mult)
            return anorm

        def make_diag(anorm, n, on_pool):
            dg = diag_pool.tile([P, P], FP16, name="dg", tag="dg")
            if on_pool:
                nc.gpsimd.tensor_tensor(
                    out=dg[:], in0=id16[:],
                    in1=anorm[:, n:n + 1].broadcast_to([P, P]), op=ALU.mult)
            else:
                nc.vector.tensor_scalar(out=dg[:], in0=id16[:],
                                        scalar1=anorm[:, n:n + 1],
                                        scalar2=1.0,
                                        op0=ALU.mult, op1=ALU.mult)
            return dg

        for ci in range(NCK):
            b, c = divmod(ci, NCHUNK)
            t0 = c * P
            # ---- R(ci): loads ----
            vslices = []
            for q in range(N):
                vt = v_pool.tile([P, D], FP16, name=f"vs{q}", tag=f"vs{q}")
                nc.sync.dma_start(vt[:], v_in[q, b, t0:t0 + P, :])
                vslices.append(vt)
            if ci == 0:
                nc.gpsimd.memset(id16[:], 1.0)
                nc.gpsimd.affine_select(out=id16[:], in_=id16[:],
                                        pattern=[[1, P]],
                                        compare_op=ALU.is_equal, fill=0.0,
                                        base=0, channel_multiplier=-1)
            vts = [vslices[q][:] for q in range(N)]

            # ---- S(ci-1) head: smalls first in every queue ----
            if pend is not None:
                anorm = softmax_smalls(pend)
                dgs = [make_diag(anorm, n, n < N_POOL_DIAG) for n in range(N)]
                pacc = psum_pool.tile([P, D], FP32, name="acc", tag="acc")
                pout = out_pool.tile([P, D], FP16, name="out_sb", tag="ot")

            ss = small_pool.tile([P, N], FP32, name="ss", tag="ss")
            dot = small_pool.tile([P, N], FP32, name="dot", tag="dot")

            # Pool dot-product multiplies (earliest slices)
            pool_prods = {}
            for q in POOL_DOT:
                sp = scrp_pool.tile([P, D], FP16, name="scrp", tag="scrp")
                nc.gpsimd.tensor_tensor(out=sp[:], in0=vts[q], in1=wb_t[:],
                                        op=ALU.mult)
                pool_prods[q] = sp

            # ACT squares
            for q in range(N):
                if q in DVE_SQ:
                    continue
                nc.scalar.activation(scr_act[:], vts[q], AF.Square,
                                     accum_out=ss[:, q:q + 1])
            # PE: warms (paced by loads) interleaved with prev chunk's MAC
            warm_ps = warm_pool.tile([P, 512], FP32, name="warm_ps", tag="wp")
            for q in range(N):
                nc.tensor.matmul(warm_ps[:], id16[:], vts[q][:, 0:512],
                                 start=True, stop=True)
                if pend is not None:
                    for h in range(2):
                        nc.tensor.matmul(pacc[:, h * 512:(h + 1) * 512],
                                         dgs[q][:],
                                         pend["vts"][q][:,
                                                        h * 512:(h + 1) * 512],
                                         start=(q == 0), stop=(q == N - 1))

            # DVE work in slice order: dots (TT+sum) + DVE squares
            for q in range(N):
                if q in POOL_DOT:
                    nc.vector.tensor_scalar(
                        out=pool_prods[q][:], in0=pool_prods[q][:],
                        scalar1=1.0, scalar2=0.0, op0=ALU.mult, op1=ALU.add,
                        accum_out=dot[:, q:q + 1])
                else:
                    nc.vector.tensor_tensor(out=scr_dve[:], in0=vts[q],
                                            in1=wb_t[:], op=ALU.mult)
                    nc.vector.tensor_scalar(
                        out=scr_dve[:], in0=scr_dve[:],
                        scalar1=1.0, scalar2=0.0, op0=ALU.mult, op1=ALU.add,
                        accum_out=dot[:, q:q + 1])
                if q in DVE_SQ:
                    nc.vector.tensor_tensor(out=scr_dve[:], in0=vts[q],
                                            in1=vts[q], op=ALU.mult)
                    nc.vector.tensor_scalar(
                        out=scr_dve[:], in0=scr_dve[:],
                        scalar1=1.0, scalar2=0.0, op0=ALU.mult, op1=ALU.add,
                        accum_out=ss[:, q:q + 1])

            # ---- S(ci-1) tail: drain + store ----
            if pend is not None:
                ac = ACT_DRAIN_COLS
                with tc.high_priority(offset=-100):
                    if ac > 0:
                        nc.scalar.activation(pout[:, 0:ac], pacc[:, 0:ac],
                                             AF.Copy)
                    if ac < D:
                        nc.vector.tensor_copy(pout[:, ac:D], pacc[:, ac:D])
                stores.append((out_d[pend["b"], pend["t0"]:pend["t0"] + P, :],
                               pout[:]))

            pend = {"ss": ss, "dot": dot, "vts": vts, "b": b, "t0": t0}

        # ---- S(last): softmax + MAC + 3-piece drain, PE kept hot ----
        anorm = softmax_smalls(pend)
        dgs = [make_diag(anorm, n, n < N_POOL_DIAG) for n in range(N)]
        warm_ps = warm_pool.tile([P, 512], FP32, name="warm_ps", tag="wp")
        for _ in range(13):
            nc.tensor.matmul(warm_ps[:], id16[:],
                             pend["vts"][N - 1][:, 0:512],
                             start=True, stop=True)
        out_sb = out_pool.tile([P, D], FP16, name="out_sb", tag="ot")
        for a0, a1 in ((0, 512), (512, 768), (768, 1024)):
            w = a1 - a0
            accl = psum_pool.tile([P, 512], FP32, name="accl", tag="accl")
            for n in range(N):
                nc.tensor.matmul(accl[:, 0:w], dgs[n][:],
                                 pend["vts"][n][:, a0:a1],
                                 start=(n == 0), stop=(n == N - 1))
            if a0 == 0:
                nc.scalar.activation(out_sb[:, a0:a1], accl[:, 0:w], AF.Copy)
            else:
                nc.vector.tensor_copy(out_sb[:, a0:a1], accl[:, 0:w])
            stores.append((out_d[pend["b"], pend["t0"]:pend["t0"] + P, a0:a1],
                           out_sb[:, a0:a1]))

        # all stores issue after the last load in SP program order
        for dst, src in stores:
            nc.sync.dma_start(dst, src)
    nc.compile()
    return nc


_NC = None


def _get_nc() -> bacc.Bacc:
    global _NC
    if _NC is None:
        _NC = _build_nc()
    return _NC


def _make_in_maps(V, w_l, norm_weight):
    V16 = np.asarray(V).astype(np.float16)
    w = np.asarray(w_l, np.float32) * np.asarray(norm_weight, np.float32)
    wb = np.ascontiguousarray(w.astype(np.float16).reshape(2, D // 2))
    ones = np.ones((2, P), dtype=np.float16)
    in_maps = []
    for c in range(NCORES):
        vs = np.ascontiguousarray(V16[:, :, c * TSH:(c + 1) * TSH, :])
        in_maps.append({"v": vs, "wb": wb, "ones": ones})
    return in_maps


def _run(in_maps, trace=False, **kwargs):
    return run_bass_kernel_spmd(_get_nc(), in_maps, list(range(NCORES)),
                                trace=trace, **kwargs)


def kernel(V, w_l, norm_weight):
    res = _run(_make_in_maps(V, w_l, norm_weight))
    outs = [res.results[i]["out"] for i in range(NCORES)]
    return np.concatenate(outs, axis=1).astype(np.float32)


# revision 19
# speedup vs baseline: 1.2273x; 1.1972x over previous
"""Trainium2 Bass kernel for nn_AttnResLayer (sparse_attention).

Computes, for V [N=12, B=4, T=2048, D=1024] fp32:
  K = rmsnorm(V) * norm_weight
  logits[n,b,t] = dot(w_l, K[n,b,t,:])
  alpha = softmax(logits, axis=n)
  out[b,t,d] = sum_n alpha[n,b,t] * V[n,b,t,d]

Sharding: T split across 8 cores (256 tokens/core per b); w_l/norm_weight
replicated (folded into one weight row host-side). No collectives.

fp16 transport: V is cast to fp16 on the host (and the output returned as
fp16, upcast on the host), halving HBM traffic vs fp32 — the op only needs
rel_err < 2e-2 and the fp16 pipeline measures ~1.1e-2 end to end. All
reductions accumulate in fp32 on-chip.

Three-stage software pipeline over 128-token chunks: in period i the
engines interleave R(i) (loads + squares/dots), S(i-1) (softmax smalls,
diags, MAC) and D(i-2) (PSUM drain). Queue orders avoid head-blocking:
each cross-engine-dependent small has independent work ahead of it in its
queue, and the pool-product sums sit at the points where the Pool TTs
actually complete. The MAC runs as one dense 24-matmul burst after the
8th pacing warm so it executes at full PE clock (p-state ramped by the
per-slice warms), with the two DVE-built diags (n=10,11) ordered first.

Per-chunk engine balance (12 slices of [128, 1024] fp16):
  ACT : 9 squares (Square + fused fp32 accum, 1225ns) + ln/exp smalls +
        the whole PSUM drain (Copy, 1038ns)
  DVE : 9 dot TT(v,wb)@2x + 12 tensor_scalar-sums@4x + 3 square TT+sum
        pairs + softmax smalls + diags 10,11
  Pool: 3 dot TT multiplies (slices 2,6,9) + 10 diag TTs
  PE  : dense MAC burst + 12 p-state pacing warms
All activations pinned to the table set holding ln+exp+square+copy.
All output stores issue after the last load so the fp16 store traffic
covers the final chunk's compute tail.
"""

import numpy as np
from contextlib import ExitStack

import concourse.bass as bass
import concourse.bacc as bacc
import concourse.tile as tile
from concourse import mybir
from concourse.bass_utils import run_bass_kernel_spmd


def _pinned_tables(arch, _orig=bacc.get_activation_tables):
    tables = _orig(arch)
    keep = "natural_log_exp_and_others"
    return {k: (v if k == keep else set()) for k, v in tables.items()}


N, B, T, D = 12, 4, 2048, 1024
NCORES = 8
TSH = T // NCORES
P = 128
NCHUNK = TSH // P
NCK = B * NCHUNK
EPS = 1e-6
FP32 = mybir.dt.float32
FP16 = mybir.dt.float16
AF = mybir.ActivationFunctionType
ALU = mybir.AluOpType

POOL_DOT = (2, 6, 9)      # dot-product TT multiplies on Pool
DVE_SQ = (9, 10, 11)      # squares on DVE (TT+sum); rest on ACT
N_POOL_DIAG = 10          # diags 0..9 on Pool; 10,11 on DVE
MAC_ORDER = (10, 11, 0, 1, 2, 3, 4, 5, 6, 7, 8, 9)
MAC_AFTER_WARM = 7        # dense MAC burst after this many warms


def _build_nc() -> bacc.Bacc:
    nc = bacc.Bacc("TRN2", target_bir_lowering=False, debug=False,
                   num_devices=NCORES)
    v_in = nc.dram_tensor("v", [N, B, TSH, D], FP16, kind="ExternalInput").ap()
    wb_in = nc.dram_tensor("wbt", [P, D], FP16, kind="ExternalInput").ap()
    ones_in = nc.dram_tensor("idm", [P, P], FP16, kind="ExternalInput").ap()
    out_d = nc.dram_tensor("out", [B, TSH, D], FP16, kind="ExternalOutput").ap()

    orig_tables = bacc.get_activation_tables
    bacc.get_activation_tables = _pinned_tables
    try:
        _build_body(nc, v_in, wb_in, ones_in, out_d)
    finally:
        bacc.get_activation_tables = orig_tables
    return nc


def _build_body(nc, v_in, wb_in, ones_in, out_d):
    with tile.TileContext(nc) as tc, ExitStack() as ctx:
        const_pool = ctx.enter_context(tc.tile_pool(name="const", bufs=1))
        v_pool = ctx.enter_context(tc.tile_pool(name="vp", bufs=3))
        scr_pool = ctx.enter_context(tc.tile_pool(name="scr", bufs=1))
        scrp_pool = ctx.enter_context(tc.tile_pool(name="scrp", bufs=3))
        small_pool = ctx.enter_context(tc.tile_pool(name="small", bufs=3))
        diag_pool = ctx.enter_context(tc.tile_pool(name="diag", bufs=24))
        psum_pool = ctx.enter_context(
            tc.tile_pool(name="accp", bufs=2, space="PSUM"))
        warm_pool = ctx.enter_context(
            tc.tile_pool(name="warmp", bufs=1, space="PSUM"))
        out_pool = ctx.enter_context(tc.tile_pool(name="outp", bufs=8))

        eps_t = const_pool.tile([P, 1], FP32, name="eps_t")
        nc.vector.memset(eps_t[:], EPS)
        id16 = const_pool.tile([P, P], FP16, name="id16")
        nc.scalar.dma_start(id16[:], ones_in[:])
        wb_t = const_pool.tile([P, D], FP16, name="wb_t")
        nc.scalar.dma_start(wb_t[:], wb_in[:])
        scr_act = scr_pool.tile([P, D], FP16, name="scr_act")
        scr_dve = scr_pool.tile([P, D], FP16, name="scr_dve")

        stores = []
        pend = None   # chunk awaiting S (reductions done)
        macd = None   # chunk awaiting D (MAC done, drain pending)

        def dve_sq(q, vts, ss):
            nc.vector.tensor_tensor(out=scr_dve[:], in0=vts[q],
                                    in1=vts[q], op=ALU.mult)
            nc.vector.tensor_scalar(
                out=scr_dve[:], in0=scr_dve[:], scalar1=1.0, scalar2=0.0,
                op0=ALU.mult, op1=ALU.add, accum_out=ss[:, q:q + 1])

        def dve_dot(q, vts, dot):
            nc.vector.tensor_tensor(out=scr_dve[:], in0=vts[q],
                                    in1=wb_t[:], op=ALU.mult)
            nc.vector.tensor_scalar(
                out=scr_dve[:], in0=scr_dve[:], scalar1=1.0, scalar2=0.0,
                op0=ALU.mult, op1=ALU.add, accum_out=dot[:, q:q + 1])

        def drain(st, last_piece=None):
            with tc.high_priority(offset=-100):
                nc.scalar.activation(st["out"][:], st["acc"][:], AF.Copy)
            stores.append((out_d[st["b"], st["t0"]:st["t0"] + P, :],
                           st["out"][:]))

        for ci in range(NCK):
            b, c = divmod(ci, NCHUNK)
            t0 = c * P
            # ---- R(ci): loads ----
            vslices = []
            for q in range(N):
                vt = v_pool.tile([P, D], FP16, name=f"vs{q}", tag=f"vs{q}")
                nc.sync.dma_start(vt[:], v_in[q, b, t0:t0 + P, :])
                vslices.append(vt)
            vts = [vslices[q][:] for q in range(N)]

            ss = small_pool.tile([P, N], FP32, name="ss", tag="ss")
            dot = small_pool.tile([P, N], FP32, name="dot", tag="dot")
            pool_prods = {}

            def pool_tt(q):
                sp = scrp_pool.tile([P, D], FP16, name="scrp", tag="scrp")
                nc.gpsimd.tensor_tensor(out=sp[:], in0=vts[q], in1=wb_t[:],
                                        op=ALU.mult)
                pool_prods[q] = sp

            def pool_tsum(q):
                nc.vector.tensor_scalar(
                    out=pool_prods[q][:], in0=pool_prods[q][:],
                    scalar1=1.0, scalar2=0.0, op0=ALU.mult, op1=ALU.add,
                    accum_out=dot[:, q:q + 1])

            # ACT head: Ln/Exp(ci-1) — ss(ci-1) complete, never blocks
            if pend is not None:
                u = small_pool.tile([P, N], FP32, name="u", tag="u")
                nc.scalar.activation(u[:], pend["ss"][:], AF.Ln,
                                     bias=eps_t[:, 0:1], scale=1.0 / D)
                rms = small_pool.tile([P, N], FP32, name="rms", tag="rms")
                nc.scalar.activation(rms[:], u[:], AF.Exp, scale=-0.5)
            # DVE head: logits smalls
            if pend is not None:
                logits = small_pool.tile([P, N], FP32, name="lg", tag="lg")
                nc.vector.tensor_mul(logits[:], pend["dot"][:], rms[:])
                negmax = small_pool.tile([P, 1], FP32, name="nm", tag="nm")
                nc.vector.tensor_reduce(negmax[:], logits[:],
                                        axis=mybir.AxisListType.X,
                                        op=ALU.max, negate=True)
            # ACT: first square, then aexp (no accum; sum on DVE)
            act_sq = [q for q in range(N) if q not in DVE_SQ]
            nc.scalar.activation(scr_act[:], vts[act_sq[0]], AF.Square,
                                 accum_out=ss[:, act_sq[0]:act_sq[0] + 1])
            if pend is not None:
                aexp = small_pool.tile([P, N], FP32, name="ax", tag="ax")
                nc.scalar.activation(aexp[:], logits[:], AF.Exp,
                                     bias=negmax[:, 0:1])
            # DVE: dot(0) fills the gap, then finish the alpha chain
            dve_dots = [q for q in range(N) if q not in POOL_DOT]
            dve_dot(dve_dots[0], vts, dot)
            if pend is not None:
                sumexp = small_pool.tile([P, 1], FP32, name="se", tag="se")
                nc.vector.tensor_scalar(
                    out=aexp[:], in0=aexp[:], scalar1=1.0, scalar2=0.0,
                    op0=ALU.mult, op1=ALU.add, accum_out=sumexp[:])
                recip = small_pool.tile([P, 1], FP32, name="rc", tag="rc")
                nc.vector.reciprocal(recip[:], sumexp[:])
                anorm = small_pool.tile([P, N], FP32, name="an", tag="an")
                nc.vector.tensor_scalar(out=anorm[:], in0=aexp[:],
                                        scalar1=recip[:, 0:1], scalar2=1.0,
                                        op0=ALU.mult, op1=ALU.mult)
                dgs = {}
                for n in range(N_POOL_DIAG, N):
                    dg = diag_pool.tile([P, P], FP16, name="dg", tag="dg")
                    nc.vector.tensor_scalar(out=dg[:], in0=id16[:],
                                            scalar1=anorm[:, n:n + 1],
                                            scalar2=1.0,
                                            op0=ALU.mult, op1=ALU.mult)
                    dgs[n] = dg
                for n in range(N_POOL_DIAG):
                    dg = diag_pool.tile([P, P], FP16, name="dg", tag="dg")
                    nc.gpsimd.tensor_tensor(
                        out=dg[:], in0=id16[:],
                        in1=anorm[:, n:n + 1].broadcast_to([P, P]),
                        op=ALU.mult)
                    dgs[n] = dg
                pacc = psum_pool.tile([P, D], FP32, name="acc", tag="acc")
                pout = out_pool.tile([P, D], FP16, name="out_sb", tag="ot")

            # ACT: next square, drain(ci-2), rest of squares
            nc.scalar.activation(scr_act[:], vts[act_sq[1]], AF.Square,
                                 accum_out=ss[:, act_sq[1]:act_sq[1] + 1])
            if macd is not None:
                drain(macd)
                macd = None
            for q in act_sq[2:]:
                nc.scalar.activation(scr_act[:], vts[q], AF.Square,
                                     accum_out=ss[:, q:q + 1])

            # Pool: dot TTs after the diags(ci-1)
            for q in POOL_DOT:
                pool_tt(q)

            # PE: dense MAC burst paced only by diag arrival; two late
            # pacing matmuls on the Pool products bridge the period
            # boundary so the p-state never sees a long cold idle.
            warm_ps = warm_pool.tile([P, 512], FP32, name="warm_ps", tag="wp")
            if ci == 0:
                for k in range(N):
                    nc.tensor.matmul(warm_ps[:], id16[:], vts[k][:, 0:512],
                                     start=True, stop=True)
            if pend is not None:
                for k, n in enumerate(MAC_ORDER):
                    for h in range(2):
                        nc.tensor.matmul(pacc[:, h * 512:(h + 1) * 512],
                                         dgs[n][:],
                                         pend["vts"][n][:,
                                                        h * 512:(h + 1) * 512],
                                         start=(k == 0), stop=(k == N - 1))

            # DVE: remaining dots + squares, pool sums at readiness points
            for q in dve_dots[1:5]:
                dve_dot(q, vts, dot)
            pool_tsum(POOL_DOT[0])
            for q in dve_dots[5:7]:
                dve_dot(q, vts, dot)
            dve_sq(9, vts, ss)
            dve_dot(dve_dots[7], vts, dot)
            dve_sq(10, vts, ss)
            pool_tsum(POOL_DOT[1])
            nc.tensor.matmul(warm_ps[:], id16[:],
                             pool_prods[POOL_DOT[1]][:, 0:512],
                             start=True, stop=True)
            dve_dot(dve_dots[8], vts, dot)
            dve_sq(11, vts, ss)
            pool_tsum(POOL_DOT[2])
            nc.tensor.matmul(warm_ps[:], id16[:],
                             pool_prods[POOL_DOT[2]][:, 0:512],
                             start=True, stop=True)

            if pend is not None:
                macd = {"acc": pacc, "out": pout,
                        "b": pend["b"], "t0": pend["t0"]}
            pend = {"ss": ss, "dot": dot, "vts": vts, "b": b, "t0": t0}

        # ---- tail: S(NCK-1) smalls first, then drain(NCK-2) ----
        u = small_pool.tile([P, N], FP32, name="u", tag="u")
        nc.scalar.activation(u[:], pend["ss"][:], AF.Ln,
                             bias=eps_t[:, 0:1], scale=1.0 / D)
        rms = small_pool.tile([P, N], FP32, name="rms", tag="rms")
        nc.scalar.activation(rms[:], u[:], AF.Exp, scale=-0.5)
        logits = small_pool.tile([P, N], FP32, name="lg", tag="lg")
        nc.vector.tensor_mul(logits[:], pend["dot"][:], rms[:])
        negmax = small_pool.tile([P, 1], FP32, name="nm", tag="nm")
        nc.vector.tensor_reduce(negmax[:], logits[:],
                                axis=mybir.AxisListType.X,
                                op=ALU.max, negate=True)
        aexp = small_pool.tile([P, N], FP32, name="ax", tag="ax")
        sumexp = small_pool.tile([P, 1], FP32, name="se", tag="se")
        nc.scalar.activation(aexp[:], logits[:], AF.Exp,
                             bias=negmax[:, 0:1], accum_out=sumexp[:])
        recip = small_pool.tile([P, 1], FP32, name="rc", tag="rc")
        nc.vector.reciprocal(recip[:], sumexp[:])
        anorm = small_pool.tile([P, N], FP32, name="an", tag="an")
        nc.vector.tensor_scalar(out=anorm[:], in0=aexp[:],
                                scalar1=recip[:, 0:1], scalar2=1.0,
                                op0=ALU.mult, op1=ALU.mult)
        if macd is not None:
            drain(macd)
        warm_ps = warm_pool.tile([P, 512], FP32, name="warm_ps", tag="wp")
        for _ in range(14):
            nc.tensor.matmul(warm_ps[:], id16[:],
                             pend["vts"][N - 1][:, 0:512],
                             start=True, stop=True)
        dgs = {}
        for n in range(N):
            dg = diag_pool.tile([P, P], FP16, name="dg", tag="dg")
            if n % 2 == 0:
                nc.gpsimd.tensor_tensor(
                    out=dg[:], in0=id16[:],
                    in1=anorm[:, n:n + 1].broadcast_to([P, P]), op=ALU.mult)
            else:
                nc.vector.tensor_scalar(out=dg[:], in0=id16[:],
                                        scalar1=anorm[:, n:n + 1],
                                        scalar2=1.0,
                                        op0=ALU.mult, op1=ALU.mult)
            dgs[n] = dg
        out_sb = out_pool.tile([P, D], FP16, name="out_sb", tag="ot")
        for a0, a1 in ((0, 512), (512, 768), (768, 1024)):
            w = a1 - a0
            accl = psum_pool.tile([P, 512], FP32, name="accl", tag="accl")
            for n in range(N):
                nc.tensor.matmul(accl[:, 0:w], dgs[n][:],
                                 pend["vts"][n][:, a0:a1],
                                 start=(n == 0), stop=(n == N - 1))
            if a0 == 0:
                nc.scalar.activation(out_sb[:, a0:a1], accl[:, 0:w], AF.Copy)
            else:
                nc.vector.tensor_copy(out_sb[:, a0:a1], accl[:, 0:w])
            stores.append((out_d[pend["b"], pend["t0"]:pend["t0"] + P, a0:a1],
                           out_sb[:, a0:a1]))

        for dst, src in stores:
            nc.sync.dma_start(dst, src)
    nc.compile()
    return nc


_NC = None


def _get_nc() -> bacc.Bacc:
    global _NC
    if _NC is None:
        _NC = _build_nc()
    return _NC


def _make_in_maps(V, w_l, norm_weight):
    V16 = np.asarray(V).astype(np.float16)
    w = np.asarray(w_l, np.float32) * np.asarray(norm_weight, np.float32)
    wbt = np.ascontiguousarray(np.broadcast_to(w.astype(np.float16), (P, D)))
    idm = np.eye(P, dtype=np.float16)
    in_maps = []
    for c in range(NCORES):
        vs = np.ascontiguousarray(V16[:, :, c * TSH:(c + 1) * TSH, :])
        in_maps.append({"v": vs, "wbt": wbt, "idm": idm})
    return in_maps


def _run(in_maps, trace=False, **kwargs):
    return run_bass_kernel_spmd(_get_nc(), in_maps, list(range(NCORES)),
                                trace=trace, **kwargs)


def kernel(V, w_l, norm_weight):
    res = _run(_make_in_maps(V, w_l, norm_weight))
    outs = [res.results[i]["out"] for i in range(NCORES)]
    return np.concatenate(outs, axis=1).astype(np.float32)
